# revision 5
# baseline (speedup 1.0000x reference)
"""Trainium2 Bass kernel for nn_CausalAttention (N=4096, 8 heads, DH=32).

Strategy: head-parallel across 8 NeuronCores (1 head per core).
Per core:
  - QKV projections from channels-major inputs [256, 4096] (natural layout
    is already the transposed layout the TensorEngine wants), fp32r.
  - Scores computed transposed: S^T[k, q] = K @ Q^T, in 512-query blocks,
    3 k-tiles (128 keys each) per PSUM group via row-packed K=32 matmuls.
    Diagonal sub-tiles skip their fully-masked column prefix.
  - Max-free softmax: P^T = exp(S / sqrt(32)) with strict-causal 0/1 mask
    applied post-exp (scores are O(1), so exp never overflows; reference's
    -10000 masking underflows to exactly 0 in f32, matching the 0-mask).
  - Softmax denominator folded into the PV matmul via a ones column
    appended to V (lhsT [128, 33]); P^T and V in bf16 (f32 accumulate).
  - Normalization without transposes: colsum -> [8, 64] (reshape DMA) ->
    reciprocal -> [1, 512] (reshape DMA) -> K=1 matmul replicate to
    [32, 512] PSUM -> one tensor_mul. Output stays in O^T layout
    [32, 4096] per core; host reshapes to [1, 256, 64, 64].
"""

import math

import numpy as np
import ml_dtypes

import concourse.bass as bass
import concourse.mybir as mybir
from concourse import bacc
from concourse.tile import TileContext
from concourse.bass_utils import run_bass_kernel_spmd

# Problem constants (hardcoded per harness contract).
B, CQ, CK, CH, NH, H, W = 1, 256, 256, 256, 8, 64, 64
DH = CH // NH            # 32
N = H * W                # 4096
QB = 512                 # queries per block
NQB = N // QB            # 8
KT = 128                 # keys per k-tile
NKT = N // KT            # 32
GS = 3                   # k-tiles per S-group (3 PSUM banks per group)
NG = (NKT + GS - 1) // GS  # 11 column-groups in packed kT layout
SCALE = 1.0 / math.sqrt(DH)

F32 = mybir.dt.float32
F32R = mybir.dt.float32r
BF16 = mybir.dt.bfloat16

_CACHED_NC = None


def _build():
    nc = bacc.Bacc("TRN2", target_bir_lowering=False, debug=False, num_devices=1)

    qin_d = nc.dram_tensor("qin", [CQ, N], F32, kind="ExternalInput")
    kin_d = nc.dram_tensor("kin", [CK, N], F32, kind="ExternalInput")
    wq_d = nc.dram_tensor("wqt", [CQ, 128], F32, kind="ExternalInput")
    wk_d = nc.dram_tensor("wkt", [CK, 128], F32, kind="ExternalInput")
    wv_d = nc.dram_tensor("wvt", [CK, DH], F32, kind="ExternalInput")
    bq_d = nc.dram_tensor("bqr", [128, 1], F32, kind="ExternalInput")
    bk_d = nc.dram_tensor("bkr", [128, 1], F32, kind="ExternalInput")
    bv_d = nc.dram_tensor("bvr", [128, DH], F32, kind="ExternalInput")
    out_d = nc.dram_tensor("out", [DH, N], F32, kind="ExternalOutput")

    # Strict-causal mask window: tm[kk, j] = 1.0 iff kk < j - 384; the
    # [*, 384:512] slice gives mask[kk, qq] = (kk < qq) for the 128-wide
    # diagonal window.
    tm_np = (np.arange(128)[:, None] < (np.arange(512)[None, :] - 384)).astype(
        ml_dtypes.bfloat16
    )
    tm_d = nc.inline_tensor(tm_np, name="tmask")
    ones_d = nc.inline_tensor(np.ones((1, DH), dtype=np.float32), name="onesd")

    with TileContext(nc) as tc:
        with (
            tc.tile_pool(name="constp", bufs=1) as constp,
            tc.tile_pool(name="bigp", bufs=1) as bigp,
            tc.tile_pool(name="workp", bufs=3) as workp,
            tc.tile_pool(name="spool", bufs=2, space="PSUM") as spool,
            tc.tile_pool(name="mpool", bufs=2, space="PSUM") as mpool,
        ):
            # ---- input streams first (sync HWDGE queue) ----
            kin_sb = bigp.tile([128, 2, N], F32R, name="kin_sb")
            qin_sb = bigp.tile([128, 2, N], F32R, name="qin_sb")
            kin_ap = kin_d.ap().rearrange("(c p) n -> p c n", p=128).bitcast(F32R)
            qin_ap = qin_d.ap().rearrange("(c p) n -> p c n", p=128).bitcast(F32R)
            for h in range(4):
                sl = slice(1024 * h, 1024 * (h + 1))
                for ch in range(2):
                    nc.sync.dma_start(kin_sb[:, ch, sl], kin_ap[:, ch, sl])
                for ch in range(2):
                    nc.sync.dma_start(qin_sb[:, ch, sl], qin_ap[:, ch, sl])

            # ---- constants / weights (gpsimd SWDGE queue, in parallel) ----
            tm_sb = constp.tile([128, 512], BF16, name="tm_sb")
            nc.gpsimd.dma_start(tm_sb[:], tm_d.ap())
            ones_sb = constp.tile([1, DH], F32R, name="ones_sb")
            nc.gpsimd.dma_start(ones_sb[:], ones_d.ap().bitcast(F32R))
            wq_sb = constp.tile([128, 2, 128], F32R, name="wq_sb")
            nc.gpsimd.dma_start(
                wq_sb[:], wq_d.ap().rearrange("(c p) m -> p c m", p=128).bitcast(F32R)
            )
            wk_sb = constp.tile([128, 2, 128], F32R, name="wk_sb")
            nc.gpsimd.dma_start(
                wk_sb[:], wk_d.ap().rearrange("(c p) m -> p c m", p=128).bitcast(F32R)
            )
            wv_sb = constp.tile([128, 2, DH], F32R, name="wv_sb")
            nc.gpsimd.dma_start(
                wv_sb[:], wv_d.ap().rearrange("(c p) m -> p c m", p=128).bitcast(F32R)
            )
            bq_sb = constp.tile([128, 1], F32, name="bq_sb")
            nc.gpsimd.dma_start(bq_sb[:], bq_d.ap())
            bk_sb = constp.tile([128, 1], F32, name="bk_sb")
            nc.gpsimd.dma_start(bk_sb[:], bk_d.ap())
            bv_sb = constp.tile([128, DH], F32, name="bv_sb")
            nc.gpsimd.dma_start(bv_sb[:], bv_d.ap())

            # ---- projections (interleaved per 512-slice for fast start) ----
            # kT3[32u+d, 128g+kk] = k^T[d, 128*(3g+u)+kk]  (3-way row packing)
            kT3 = bigp.tile([96, NG * 128], F32R, name="kT3")
            # qT[32u+d, q] = q^T[d, q] for u=0..3 (4x replicated on partitions)
            qT = bigp.tile([128, N], F32R, name="qT")
            # v_all[kk, t, :DH] = v[128t+kk, :]; col DH is the ones column
            v_all = bigp.tile([128, NKT, DH + 1], BF16, name="v_all")
            nc.vector.memset(v_all[:, :, DH : DH + 1], 1.0)

            for s in range(8):
                ksl = slice(512 * s, 512 * (s + 1))
                pj = mpool.tile([128, 512], F32, name="pj", tag="m")
                for ch in range(2):
                    nc.tensor.matmul(
                        pj[:],
                        wk_sb[:, ch, :],
                        kin_sb[:, ch, ksl],
                        start=(ch == 0),
                        stop=(ch == 1),
                    )
                for ci in range(4):
                    j = 4 * s + ci
                    u, g = j % GS, j // GS
                    nc.vector.tensor_scalar_add(
                        kT3[32 * u : 32 * u + 32, 128 * g : 128 * g + 128],
                        pj[32 * u : 32 * u + 32, 128 * ci : 128 * ci + 128],
                        bk_sb[32 * u : 32 * u + 32, :],
                    )
                pj = mpool.tile([128, 512], F32, name="pj", tag="m")
                for ch in range(2):
                    nc.tensor.matmul(
                        pj[:],
                        wq_sb[:, ch, :],
                        qin_sb[:, ch, ksl],
                        start=(ch == 0),
                        stop=(ch == 1),
                    )
                nc.vector.tensor_scalar_add(qT[:, ksl], pj[:], bq_sb[:])
                for t in range(4 * s, 4 * s + 4):
                    nsl = slice(128 * t, 128 * (t + 1))
                    pj = mpool.tile([128, DH], F32, name="pj", tag="m")
                    for ch in range(2):
                        nc.tensor.matmul(
                            pj[:],
                            kin_sb[:, ch, nsl],
                            wv_sb[:, ch, :],
                            start=(ch == 0),
                            stop=(ch == 1),
                        )
                    nc.vector.tensor_add(v_all[:, t, 0:DH], pj[:], bv_sb[:])

            # ---- attention over q-blocks (PV software-pipelined 1 group back) ----
            for qb in range(NQB):
                o_ps = mpool.tile([DH + 1, 512], F32, name="o_ps", tag="m")
                nkt_q = 4 * (qb + 1)          # causal: k-tiles 0..nkt_q-1
                ngr = (nkt_q + GS - 1) // GS
                pend = None                    # (g, nsub, p_sb) awaiting PV
                first = True

                def flush_pv(pend, first, last, o_ps=o_ps):
                    g, nsub, p_sb = pend
                    f = first
                    for u in range(nsub):
                        j = GS * g + u
                        nc.tensor.matmul(
                            o_ps[:],
                            v_all[:, j, :],
                            p_sb[:, 512 * u : 512 * (u + 1)],
                            start=f,
                            stop=(last and u == nsub - 1),
                            skip_group_check=True,
                        )
                        f = False
                    return False

                for g in range(ngr):
                    nsub = min(GS, nkt_q - GS * g)
                    s_ps = spool.tile([128, GS * 512], F32, name="s_ps")
                    for u in range(nsub):
                        j = GS * g + u
                        o = max(0, 128 * j - 512 * qb)
                        nc.tensor.matmul(
                            s_ps[:, 512 * u + o : 512 * (u + 1)],
                            kT3[32 * u : 32 * u + 32, 128 * g : 128 * g + 128],
                            qT[32 * u : 32 * u + 32, 512 * qb + o : 512 * (qb + 1)],
                            start=True,
                            stop=True,
                        )
                    p_sb = workp.tile([128, GS * 512], BF16, name="p_sb", bufs=6)
                    nc.scalar.activation(
                        p_sb[:, 0 : 512 * nsub],
                        s_ps[:, 0 : 512 * nsub],
                        mybir.ActivationFunctionType.Exp,
                        scale=SCALE,
                    )
                    for u in range(nsub):
                        j = GS * g + u
                        o = 128 * j - 512 * qb
                        if o > 0:  # zero the fully-masked prefix (stale exp)
                            nc.vector.memset(p_sb[:, 512 * u : 512 * u + o], 0.0)
                        if o >= 0:  # strict-causal mask on the diagonal window
                            nc.vector.tensor_mul(
                                p_sb[:, 512 * u + o : 512 * u + o + 128],
                                p_sb[:, 512 * u + o : 512 * u + o + 128],
                                tm_sb[:, 384:512],
                            )
                    if pend is not None:
                        first = flush_pv(pend, first, last=False)
                    pend = (g, nsub, p_sb)
                first = flush_pv(pend, first, last=True)

                # ---- block tail: normalize in O^T layout, no transposes ----
                o_sb = workp.tile([DH, 512], F32, name="o_sb")
                nc.vector.tensor_copy(o_sb[:], o_ps[0:DH, :])
                cs_sb = workp.tile([1, 512], F32, name="cs_sb")
                # +1e-30 keeps q=0 (fully masked row) at 0 instead of NaN
                nc.vector.tensor_scalar_add(cs_sb[:], o_ps[DH : DH + 1, :], 1e-30)
                cs8 = workp.tile([8, 64], F32, name="cs8")
                nc.sync.dma_start(cs8[:], cs_sb[:])
                cs8r = workp.tile([8, 64], F32, name="cs8r")
                nc.vector.reciprocal(cs8r[:], cs8[:])
                csr = workp.tile([1, 512], F32R, name="csr")
                nc.sync.dma_start(csr[:], cs8r[:].bitcast(F32R))
                rep_ps = mpool.tile([DH, 512], F32, name="rep_ps", tag="m")
                nc.tensor.matmul(rep_ps[:], ones_sb[:], csr[:], start=True, stop=True)
                out_sb = workp.tile([DH, 512], F32, name="out_sb")
                nc.vector.tensor_mul(out_sb[:], o_sb[:], rep_ps[:])
                nc.sync.dma_start(out_d.ap()[:, 512 * qb : 512 * (qb + 1)], out_sb[:])

    nc.finalize()
    return nc


def _get_nc():
    global _CACHED_NC
    if _CACHED_NC is None:
        _CACHED_NC = _build()
    return _CACHED_NC


def _prep_in_maps(inputs):
    f = lambda a: np.ascontiguousarray(np.asarray(a, dtype=np.float32))
    query = f(inputs["query"]).reshape(CQ, N)
    key_feat = f(inputs["key_feat"]).reshape(CK, N)

    def wnorm(v, g):
        v = f(v)
        g = f(g)
        return g[:, None] * v / np.linalg.norm(v, axis=1, keepdims=True)

    wq = wnorm(inputs["vq"], inputs["gq"])
    wk = wnorm(inputs["vk"], inputs["gk"])
    wv = wnorm(inputs["vv"], inputs["gv"])
    bq, bk, bv = f(inputs["bq"]), f(inputs["bk"]), f(inputs["bv"])

    in_maps = []
    for c in range(NH):
        rows = slice(DH * c, DH * (c + 1))
        in_maps.append(
            {
                "qin": query,
                "kin": key_feat,
                "wqt": np.ascontiguousarray(np.tile(wq[rows].T, (1, 4))),
                "wkt": np.ascontiguousarray(np.tile(wk[rows].T, (1, 4))),
                "wvt": np.ascontiguousarray(wv[rows].T),
                "bqr": np.ascontiguousarray(np.tile(bq[rows], 4)[:, None]),
                "bkr": np.ascontiguousarray(np.tile(bk[rows], 4)[:, None]),
                "bvr": np.ascontiguousarray(np.tile(bv[rows][None, :], (128, 1))),
            }
        )
    return in_maps


def _run(inputs, trace=False, **kwargs):
    nc = _get_nc()
    in_maps = _prep_in_maps(inputs)
    res = run_bass_kernel_spmd(
        nc, in_maps, core_ids=list(range(NH)), trace=trace, **kwargs
    )
    out = np.empty((B, CH, H, W), dtype=np.float32)
    for c in range(NH):
        oc = res.results[c]["out"]  # [DH, N] (O^T layout)
        out[0, DH * c : DH * (c + 1)] = oc.reshape(DH, H, W)
    return out, res


def kernel(**inputs) -> np.ndarray:
    out, _ = _run(inputs, trace=False)
    return out
